# revision 20
# baseline (speedup 1.0000x reference)
"""Trainium2 Bass kernel: GQA attention layer (RoPE + causal sliding-window)
tensor-parallel across heads on 8 NeuronCores.

Problem shapes (hardcoded): S=2048 tokens, DIM=4096, HQ=32 q-heads,
HKV=8 kv-heads, HD=128 head dim, window=2048 (window >= S, so the mask is
plain causal).

Sharding: core c owns kv-head c and q-heads 4c..4c+3 (column-parallel
wq/wk/wv, row-parallel wo). Each core computes a full [S, DIM] partial of
the output projection in f16; the host sums the 8 partials in f32.

Device-side layout notes:
 - All matmul operands are bf16 (fp32 accumulate in PSUM).
 - Projections are computed in "transposed" layout qT/kT [HD, S] directly
   (out = W^T.T @ x^T), which is what the scoresT QK matmul wants. v gets a
   DMA-transpose back to natural [S, HD].
 - The head dim of q/k is de-interleaved (even dims in partitions 0..63,
   odd in 64..127) by permuting wq/wk columns on the host. RoPE is then two
   ACT partition-swap copies + four partition-aligned DVE ops per
   [128, 512] block. Dot products are permutation-invariant, so scores are
   unchanged.
 - scoresT blocks are [kj, qi]: adjacent kj tiles are paired into one
   2-bank PSUM tile so a single ACT exp processes [128, 1024] (amortizes
   the ~350-cycle ACT op overhead; ACT no longer paces the PE in phase B).
   Diagonal blocks are computed full-width (their upper-triangle part is
   valid-but-masked scores) and causal masking is gpsimd.affine_select
   after exp; PV/denominator matmuls still skip fully-masked columns.
   Softmax denominator = ones-matmul chain (stays on PE; PE has the
   headroom there vs ACT/DVE). No max subtraction: |scores*scale| < ~7 for
   these inputs, well within fp32/exp range.
 - DMA queue plan: inputs ride sync (x, half of wq) and gpsimd
   (wk/wv/wq/wo); cos/sin + all output DMAs ride the scalar (ACT) HWDGE
   queue. In the steady-state timing loop the next iteration's input DMAs
   therefore overlap this iteration's phase-C output drain.
"""

from contextlib import ExitStack, nullcontext

import numpy as np
import ml_dtypes

import concourse.bass as bass
import concourse.mybir as mybir
import concourse.tile as tile
from concourse import bacc
from concourse.bass_utils import run_bass_kernel_spmd

S = 2048
DIM = 4096
HQ, HKV, HD = 32, 8, 128
NCORES = 8
GH = HQ // HKV          # q heads per core (= per kv head) = 4
P = 128
KT = DIM // P           # 32 contraction tiles
SC = 512                # s-chunk (psum free dim)
NSC = S // SC           # 4
NQT = S // P            # 16 query tiles of 128
NMC = DIM // SC         # 8 output column chunks
SCALE = float(HD) ** -0.5

F32 = mybir.dt.float32
F16 = mybir.dt.float16
BF16 = mybir.dt.bfloat16

_CACHE = {}


def _build_bass(loop_n=1, staggered=False):
    """loop_n > 1 wraps the whole body in a hardware For_i loop — used only
    by the test harness for differential wall-clock timing (the axon
    dispatch floor is ~80 ms, far above the kernel's execution time)."""
    nc = bacc.Bacc("TRN2", target_bir_lowering=False, debug=False,
                   enable_asserts=False)
    xT_d = nc.dram_tensor("xt", [DIM, S], BF16, kind="ExternalInput")
    wq_d = nc.dram_tensor("wqt", [DIM, GH * HD], BF16, kind="ExternalInput")
    wk_d = nc.dram_tensor("wkt", [DIM, HD], BF16, kind="ExternalInput")
    wv_d = nc.dram_tensor("wvt", [DIM, HD], BF16, kind="ExternalInput")
    wo_d = nc.dram_tensor("wot", [GH * HD, DIM], BF16, kind="ExternalInput")
    cos_d = nc.dram_tensor("cos2", [P, S], BF16, kind="ExternalInput")
    sin_d = nc.dram_tensor("sin2", [P, S], BF16, kind="ExternalInput")
    out_d = nc.dram_tensor("out", [S, DIM], F16, kind="ExternalOutput")

    with tile.TileContext(nc) as tc, ExitStack() as ctx:
        consts = ctx.enter_context(tc.tile_pool(name="consts", bufs=1))
        state = ctx.enter_context(tc.tile_pool(name="state", bufs=1))
        xpool = ctx.enter_context(tc.tile_pool(name="xpool", bufs=2))
        ropep = ctx.enter_context(tc.tile_pool(name="ropep", bufs=2))
        expp = ctx.enter_context(tc.tile_pool(name="expp", bufs=6))
        osb = ctx.enter_context(tc.tile_pool(name="osb", bufs=2))
        rcp = ctx.enter_context(tc.tile_pool(name="rcp", bufs=2))
        vtp = ctx.enter_context(tc.tile_pool(name="vtp", bufs=2))
        # PSUM (8 banks): phase A projection chains and phase C wo chains
        # share mm_ps (2 banks); scores get 2x 2-bank tiles (paired exp);
        # pv+dn accumulators share 2 banks.
        mm_ps = ctx.enter_context(tc.tile_pool(name="mm_ps", bufs=2, space="PSUM"))
        sc_ps = ctx.enter_context(tc.tile_pool(name="sc_ps", bufs=2, space="PSUM"))
        acc_ps = ctx.enter_context(tc.tile_pool(name="acc_ps", bufs=2, space="PSUM"))

        # staggered_reset: stages (input+A01 | A23 | B | C) pipeline across
        # loop iterations — iteration i+1's input DMAs overlap iteration
        # i's attention/output phases instead of a full-barrier back edge.
        ALL_ENGINES = (mybir.EngineType.PE, mybir.EngineType.DVE,
                       mybir.EngineType.Activation, mybir.EngineType.Pool,
                       mybir.EngineType.SP)
        loop_cm = (tc.For_i(0, loop_n, 1, staggered_reset=staggered,
                            hint_engines=ALL_ENGINES)
                   if loop_n > 1 else nullcontext())
        loop_cm.__enter__()

        def stage_boundary():
            if loop_n > 1 and staggered:
                tc.stage_boundary()

        # ---- constants / weights in SBUF ----
        # The first x chunk gates all compute: split it across sync+gpsimd.
        # wk/wv go first on gpsimd (the k and v chains run before q chains),
        # wq follows split across both queues. cos/sin ride the scalar
        # queue (it is otherwise idle until phase C's output DMAs).
        xc0 = xpool.tile([P, KT, SC], BF16, tag="x")
        x_engs = [nc.sync, nc.gpsimd, nc.scalar]
        for og in range(8):
            x_engs[og % 3].dma_start(
                xc0[:, 4 * og:4 * og + 4, :],
                xT_d.ap()[512 * og:512 * (og + 1), 0:SC]
                .rearrange("(o p) s -> p o s", p=P))
        wk_sb = consts.tile([P, KT, HD], BF16)
        nc.gpsimd.dma_start(wk_sb[:], wk_d.ap().rearrange("(o p) m -> p o m", p=P))
        wv_sb = consts.tile([P, KT, HD], BF16)
        nc.gpsimd.dma_start(wv_sb[:], wv_d.ap().rearrange("(o p) m -> p o m", p=P))
        wq_sb = consts.tile([P, KT, GH * HD], BF16)
        for og in range(8):
            eng = nc.sync if og % 2 == 1 else nc.gpsimd
            eng.dma_start(
                wq_sb[:, 4 * og:4 * og + 4, :],
                wq_d.ap()[512 * og:512 * (og + 1), :]
                .rearrange("(o p) m -> p o m", p=P))
        cos_sb = consts.tile([P, S], BF16)
        nc.scalar.dma_start(cos_sb[:], cos_d.ap())
        sin_sb = consts.tile([P, S], BF16)
        nc.scalar.dma_start(sin_sb[:], sin_d.ap())
        ones_sb = consts.tile([P, P], BF16)
        nc.vector.memset(ones_sb[:], 1.0)

        # state tiles
        qT_sb = state.tile([P, GH, S], BF16)     # rope'd q, permuted head dim
        kT_sb = state.tile([P, S], BF16)         # rope'd k, permuted head dim
        v_sb = state.tile([P, NQT, HD], BF16)    # v natural [s-tile, d]
        attnT_sb = state.tile([P, GH, S], BF16)  # attn out^T, standard head dim

        H = 64
        mul = mybir.AluOpType.mult
        CP = mybir.ActivationFunctionType.Copy

        def rope(ps, out_sl, sc):
            """ps: [128, 512] f32 psum, head dim de-interleaved (even dims
            at partitions 0..63, odd at 64..127). Writes bf16 out_sl."""
            cs = cos_sb[:, SC * sc:SC * (sc + 1)]
            sn = sin_sb[:, SC * sc:SC * (sc + 1)]
            Asw = ropep.tile([P, SC], F32, tag="ropeA")
            P1 = ropep.tile([P, SC], F32, tag="ropeB")
            # partition-swapped copy of ps (ACT can shift base partitions)
            nc.scalar.activation(Asw[0:H], ps[H:P], CP)
            nc.scalar.activation(Asw[H:P], ps[0:H], CP)
            nc.vector.tensor_tensor(P1[:], ps[:], cs, mul)        # e*c | o*c
            nc.vector.tensor_tensor(Asw[:], Asw[:], sn, mul)      # o*s | e*s
            nc.vector.tensor_tensor(out_sl[0:H], P1[0:H], Asw[0:H],
                                    mybir.AluOpType.subtract)
            nc.vector.tensor_tensor(out_sl[H:P], P1[H:P], Asw[H:P],
                                    mybir.AluOpType.add)

        # ---- phase A: QKV projections + RoPE, per s-chunk ----
        # Chain order k, v, q0..q3 so the first chain only needs xc + wk.
        for sc in range(NSC):
            if sc == 2:
                stage_boundary()
            if sc == 0:
                xc = xc0
            else:
                xc = xpool.tile([P, KT, SC], BF16, tag="x")
                for og in range(8):
                    nc.sync.dma_start(
                        xc[:, 4 * og:4 * og + 4, :],
                        xT_d.ap()[512 * og:512 * (og + 1),
                                  SC * sc:SC * (sc + 1)]
                        .rearrange("(o p) s -> p o s", p=P))
            ps = mm_ps.tile([P, SC], F32, tag="mm")
            for o in range(KT):
                nc.tensor.matmul(ps[:], wk_sb[:, o, :], xc[:, o, :],
                                 start=(o == 0), stop=(o == KT - 1))
            rope(ps, kT_sb[:, SC * sc:SC * (sc + 1)], sc)
            ps = mm_ps.tile([P, SC], F32, tag="mm")
            for o in range(KT):
                nc.tensor.matmul(ps[:], wv_sb[:, o, :], xc[:, o, :],
                                 start=(o == 0), stop=(o == KT - 1))
            vt = vtp.tile([P, SC], BF16, tag="vt")
            nc.scalar.activation(vt[:], ps[:], CP)
            for b in range(4):
                nc.sync.dma_start_transpose(v_sb[:, 4 * sc + b, :],
                                            vt[:, P * b:P * (b + 1)])
            for h in range(GH):
                ps = mm_ps.tile([P, SC], F32, tag="mm")
                for o in range(KT):
                    nc.tensor.matmul(ps[:], wq_sb[:, o, HD * h:HD * (h + 1)],
                                     xc[:, o, :], start=(o == 0),
                                     stop=(o == KT - 1))
                rope(ps, qT_sb[:, h, SC * sc:SC * (sc + 1)], sc)

        # wo weights ride in the xpool slots freed after the last x chunk
        # (gpsimd queue, after wq — needed only from phase C on)
        wo_sb = xpool.tile([P, GH, DIM], BF16, tag="x")
        for h in range(GH):
            nc.gpsimd.dma_start(wo_sb[:, h, :], wo_d.ap()[P * h:P * (h + 1), :])

        stage_boundary()

        # ---- phase B: attention per (query chunk, head) ----
        for qc in range(NSC):
            T = 4 * qc + 4        # causal: kj tiles 0..T-1 (always even)
            for h in range(GH):
                q_sl = qT_sb[:, h, SC * qc:SC * (qc + 1)]
                exs = []   # per kj tile: (ex_tile, sub-slot, column offset)
                for j in range(T // 2):
                    diag = (2 * j >= 4 * qc)
                    sps = sc_ps.tile([P, 2, SC], F32, tag="sc")
                    ex = expp.tile([P, 2, SC], BF16, tag="exp")
                    for s_ in range(2):
                        t = 2 * j + s_
                        # diagonal blocks: columns qi < 128*(t-4qc) are
                        # fully masked - compute only the suffix
                        off = max(0, P * (t - 4 * qc))
                        nc.tensor.matmul(sps[:, s_, off:],
                                         kT_sb[:, P * t:P * (t + 1)],
                                         q_sl[:, off:], start=True, stop=True)
                        exs.append((ex, s_, off))
                    if not diag:
                        # paired [128, 1024] exp amortizes the ACT op
                        # overhead (full tiles only — psum fully written)
                        nc.scalar.activation(ex[:], sps[:],
                                             mybir.ActivationFunctionType.Exp,
                                             scale=SCALE)
                        continue
                    for s_ in range(2):
                        t = 2 * j + s_
                        off = max(0, P * (t - 4 * qc))
                        nc.scalar.activation(ex[:, s_, off:],
                                             sps[:, s_, off:],
                                             mybir.ActivationFunctionType.Exp,
                                             scale=SCALE)
                        if off > 0:
                            # fully-masked cols: exact zeros so the
                            # denominator tree-sum below reads no
                            # uninitialized SBUF
                            nc.gpsimd.memset(ex[:, s_, 0:off], 0.0)
                        # keep iff (y + off + 512*qc) - (128*t + x) >= 0
                        nc.gpsimd.affine_select(
                            out=ex[:, s_, off:], in_=ex[:, s_, off:],
                            compare_op=mybir.AluOpType.is_ge,
                            fill=0.0, base=SC * qc + off - P * t,
                            pattern=[[1, SC - off]], channel_multiplier=-1)
                pv = acc_ps.tile([P, SC], F32, tag="acc")
                for t in range(T):
                    ex, s_, off = exs[t]
                    nc.tensor.matmul(pv[:, off:], v_sb[:, t, :],
                                     ex[:, s_, off:],
                                     start=(t == 0), stop=(t == T - 1))
                # denominator: in-place pairwise tree-sum of the exp pair
                # tiles on DVE/Pool (masked cols are exact zeros), then one
                # ones-matmul broadcasts the per-qi sum to 128 partitions.
                # Replaces a T-matmul ones-chain per group on the PE.
                ptiles = [exs[2 * j][0] for j in range(T // 2)]
                m = len(ptiles)
                add = mybir.AluOpType.add
                stride, eng_i = 1, 0
                while stride < m:
                    for a in range(0, m - stride, 2 * stride):
                        eng = nc.vector if eng_i % 2 == 0 else nc.gpsimd
                        eng_i += 1
                        eng.tensor_tensor(ptiles[a][:], ptiles[a][:],
                                          ptiles[a + stride][:], add)
                    stride *= 2
                dnf = rcp.tile([P, SC], BF16, tag="dnf")
                nc.vector.tensor_tensor(dnf[:], ptiles[0][:, 0, :],
                                        ptiles[0][:, 1, :], add)
                dn = acc_ps.tile([P, SC], F32, tag="acc")
                nc.tensor.matmul(dn[:], ones_sb[:], dnf[:],
                                 start=True, stop=True)
                rc = rcp.tile([P, SC], F32, tag="rc")
                nc.vector.reciprocal(rc[:], dn[:])
                nc.vector.tensor_tensor(
                    attnT_sb[:, h, SC * qc:SC * (qc + 1)], pv[:], rc[:], mul)

        stage_boundary()

        # ---- phase C: row-parallel wo projection (partial output) ----
        # Per query tile: 8 psum chains -> one [128, 4096] f16 staging tile
        # -> one 1 MB DMA on the scalar queue.
        for qt in range(NQT):
            ostg = osb.tile([P, DIM], F16, tag="ob")
            for mc in range(NMC):
                # alternate between both psum pools (acc is idle in phase C)
                # so four banks rotate and the psum->staging copies hide
                pool = mm_ps if mc % 2 == 0 else acc_ps
                wps = pool.tile([P, SC], F32, tag="mm" if mc % 2 == 0 else "acc")
                for h in range(GH):
                    nc.tensor.matmul(wps[:],
                                     attnT_sb[:, h, P * qt:P * (qt + 1)],
                                     wo_sb[:, h, SC * mc:SC * (mc + 1)],
                                     start=(h == 0), stop=(h == GH - 1))
                dst = ostg[:, SC * mc:SC * (mc + 1)]
                if mc % 2 == 0:
                    nc.vector.tensor_copy(dst, wps[:])
                else:
                    nc.scalar.activation(dst, wps[:], CP)
            nc.scalar.dma_start(out_d.ap()[P * qt:P * (qt + 1), :], ostg[:])

        loop_cm.__exit__(None, None, None)

    nc.compile()
    return nc


# head-dim de-interleave permutation: [0,2,...,126, 1,3,...,127]
_PERM = np.concatenate([np.arange(0, HD, 2), np.arange(1, HD, 2)])


def _prep_inputs(x, wq, wk, wv, wo, cos, sin):
    """Host-side shard + layout prep. Returns list of 8 per-core input maps."""
    bf = ml_dtypes.bfloat16
    xT = np.ascontiguousarray(x.T.astype(bf))
    # cos/sin tables duplicated across both 64-partition halves
    cosT = np.ascontiguousarray(cos.T.astype(bf))           # [64, S]
    sinT = np.ascontiguousarray(sin.T.astype(bf))
    cos2 = np.concatenate([cosT, cosT], axis=0)             # [128, S]
    sin2 = np.concatenate([sinT, sinT], axis=0)
    in_maps = []
    for c in range(NCORES):
        wq_c = wq[GH * HD * c:GH * HD * (c + 1)]            # [512, DIM]
        # de-interleave head dim within each head
        wq_cp = wq_c.reshape(GH, HD, DIM)[:, _PERM, :].reshape(GH * HD, DIM)
        wk_cp = wk[HD * c:HD * (c + 1)][_PERM, :]           # [128, DIM]
        wv_c = wv[HD * c:HD * (c + 1)]                      # [128, DIM] (no perm)
        wo_c = wo[:, GH * HD * c:GH * HD * (c + 1)]         # [DIM, 512]
        in_maps.append({
            "xt": xT,
            "wqt": np.ascontiguousarray(wq_cp.T.astype(bf)),
            "wkt": np.ascontiguousarray(wk_cp.T.astype(bf)),
            "wvt": np.ascontiguousarray(wv_c.T.astype(bf)),
            "wot": np.ascontiguousarray(wo_c.T.astype(bf)),
            "cos2": cos2,
            "sin2": sin2,
        })
    return in_maps


def kernel(x, wq, wk, wv, wo, cos, sin, window):
    assert int(window) >= S, "kernel hardcodes window >= S (plain causal)"
    x = np.asarray(x, dtype=np.float32)
    wq = np.asarray(wq, dtype=np.float32)
    wk = np.asarray(wk, dtype=np.float32)
    wv = np.asarray(wv, dtype=np.float32)
    wo = np.asarray(wo, dtype=np.float32)
    cos = np.asarray(cos, dtype=np.float32)
    sin = np.asarray(sin, dtype=np.float32)

    if "nc" not in _CACHE:
        _CACHE["nc"] = _build_bass()
    nc = _CACHE["nc"]
    in_maps = _prep_inputs(x, wq, wk, wv, wo, cos, sin)
    res = run_bass_kernel_spmd(nc, in_maps, core_ids=list(range(NCORES)))
    total = res.results[0]["out"].astype(np.float32)
    for c in range(1, NCORES):
        total += res.results[c]["out"].astype(np.float32)
    return total
